# revision 2
# baseline (speedup 1.0000x reference)
"""CARAFE-naive 2x content-aware upsampling on 8 Trainium2 NeuronCores.

Problem: features [2, 256, 100, 100] f32, masks [2, 25, 200, 200] f32
-> out [2, 256, 200, 200] f32, where each output pixel is a 25-tap (5x5)
weighted sum of the source neighborhood, weights shared across channels.

Strategy (per core = one (image n, row-quarter q) pair):
  The 25-tap contraction is cast as TensorE matmuls via a banded-matrix
  trick along the width axis. For one low-res output row h and width
  block of L=50 low-res columns, the contraction over the 5 horizontal
  taps is a matmul with contraction dim K = L+4 = 54 (the padded width
  window): out[c, (a, w2)] = sum_w' F[w', c] * Band[w', (a, w2)], where
  Band packs mask values on 5 diagonals (built host-side in numpy).
  The 5 vertical taps (dy) accumulate in PSUM across 5 matmuls.

  lhsT = transposed feature row slices (stationary), rhs = banded mask
  blocks. Both fp16 (PE runs fp16 at full rate; ~2^-11 rel precision).
  Both width blocks live on SBUF partitions [0, 54) with the block index
  in the free dim -- all matmuls use tile_position (0,0); mixing row
  bases within one PSUM accumulation group crashes the device.

Host-side numpy does layout/packing only (transpose, pad, diagonal
scatter of masks into band matrices); all FLOPs run on the device.
"""

import numpy as np

import concourse.mybir as mybir
import concourse.tile as tile
from concourse import bacc
from concourse.bass_utils import run_bass_kernel_spmd

# problem constants
N, C, H, W = 2, 256, 100, 100
KS = 5        # kernel size
S = 2         # upsample scale
R = (KS - 1) // 2

# sharding / blocking constants
HC = H // 4       # 25 low-res rows per core (8 cores = 2 images x 4 quarters)
NR = HC + 2 * R   # 29 padded feature rows per core
NBLK = 2          # width blocks
L = W // NBLK     # 50 low-res columns per block
KB = L + KS - 1   # 54 = matmul contraction size
PBASE = 64        # SBUF partition base stride between blocks
NCOL = 2 * S * L  # 200 matmul N per block: (a in 2, w2l in 100)
F16 = mybir.dt.float16
F32 = mybir.dt.float32


def build_program(iters: int = 1, dt=F16, blks=(0, 1), copy_eng="both", parts="full",
                  in_chunks: int = 1, in_engines=("sync",)):
    """Build the per-core bass program. `iters`>1 wraps the whole compute in
    a hardware loop (used only for benchmarking slope timing)."""
    nc = bacc.Bacc(None, target_bir_lowering=False, debug=False)
    f_in = nc.dram_tensor("f", [KB, NBLK, NR, C], dt, kind="ExternalInput")
    b_in = nc.dram_tensor("b", [KB, NBLK, HC, KS, NCOL], dt, kind="ExternalInput")
    out = nc.dram_tensor("out", [C, S * HC, S * W], F32, kind="ExternalOutput")

    with tile.TileContext(nc) as tc:
        with (
            tc.tile_pool(name="fsb", bufs=1) as fpool,
            tc.tile_pool(name="bsb", bufs=1) as bpool,
            tc.tile_pool(name="osb", bufs=4) as opool,
            tc.tile_pool(name="ps", bufs=6, space="PSUM") as pspool,
        ):
            def body(_=None):
                F_sb = fpool.tile([KB, NBLK, NR, C], dt)
                B_sb = bpool.tile([KB, NBLK, HC, KS, NCOL], dt)
                if parts == "dmain128":
                    # DMA-bandwidth probe: same bytes, 108-partition layout
                    F2 = fpool.tile([KB * NBLK, NR, C], dt, name="F2")
                    B2 = bpool.tile([KB * NBLK, HC, KS, NCOL], dt, name="B2")
                    f2 = f_in[:].rearrange("k n r c -> (k n) r c")
                    b2 = b_in[:].rearrange("k n h d c -> (k n) h d c")
                    engs = [getattr(nc, e) for e in in_engines]
                    step = (KB * NBLK + in_chunks - 1) // in_chunks
                    for i, p0 in enumerate(range(0, KB * NBLK, step)):
                        p1 = min(p0 + step, KB * NBLK)
                        engs[i % len(engs)].dma_start(F2[p0:p1], f2[p0:p1])
                        engs[i % len(engs)].dma_start(B2[p0:p1], b2[p0:p1])
                    return
                if parts != "nodmain":
                    engs = [getattr(nc, e) for e in in_engines]
                    ei = 0
                    # split each input DMA into in_chunks along a free dim to
                    # engage more DMA queues in parallel
                    fstep = (NR + in_chunks - 1) // in_chunks
                    for r0 in range(0, NR, fstep):
                        r1 = min(r0 + fstep, NR)
                        engs[ei % len(engs)].dma_start(
                            F_sb[:, :, r0:r1], f_in[:, :, r0:r1]
                        )
                        ei += 1
                    bstep = (HC + in_chunks - 1) // in_chunks
                    for h0 in range(0, HC, bstep):
                        h1 = min(h0 + bstep, HC)
                        engs[ei % len(engs)].dma_start(
                            B_sb[:, :, h0:h1], b_in[:, :, h0:h1]
                        )
                        ei += 1
                if parts == "dmain":
                    return
                for ct in range(2):
                    psums = {}
                    for r in range(NR):
                        for blk in blks:
                            lhsT = F_sb[:, blk, r, ct * 128 : (ct + 1) * 128]
                            for dy in range(KS):
                                h = r - dy
                                if not (0 <= h < HC):
                                    continue
                                if dy == 0 and blk == blks[0]:
                                    psums[h] = pspool.tile(
                                        [128, NBLK * NCOL],
                                        F32,
                                        name=f"ps{ct}_{h}",
                                        tag="ps",
                                    )
                                # One accumulation group per PSUM bank: start
                                # zeroes the whole 2KB zero-region, so only
                                # the first matmul of the tile starts and only
                                # the last one stops.
                                nc.tensor.matmul(
                                    psums[h][:, blk * NCOL : (blk + 1) * NCOL],
                                    lhsT,
                                    B_sb[:, blk, h, dy, :],
                                    start=(dy == 0 and blk == blks[0]),
                                    stop=(dy == KS - 1 and blk == blks[-1]),
                                )
                        h_done = r - (KS - 1)
                        if h_done >= 0 and parts in ("full", "nodmain"):
                            ps = psums.pop(h_done)
                            osb = opool.tile([128, 2, NBLK, S * L], F32)
                            # psum free layout (blk, a, w2l) -> (a, blk, w2l)
                            src = ps[:].rearrange(
                                "p (k a w) -> p a k w", k=NBLK, a=2
                            )
                            if copy_eng == "vector" or (copy_eng == "both" and h_done % 2 == 0):
                                nc.vector.tensor_copy(osb[:], src)
                            else:
                                nc.scalar.copy(osb[:], src)
                            nc.sync.dma_start(
                                out[ct * 128 : (ct + 1) * 128,
                                    S * h_done : S * h_done + 2, :],
                                osb[:].rearrange("p a k w -> p a (k w)"),
                            )

            if iters == 1:
                body()
            else:
                with tc.For_i(0, iters, 1) as _i:
                    body(_i)
    nc.finalize()
    return nc


def host_prep(features: np.ndarray, masks: np.ndarray):
    """Pack per-core fp16 inputs: transposed padded feature rows and banded
    mask matrices. Pure layout work (no arithmetic beyond dtype cast)."""
    f_hosts, b_hosts = [], []
    padded = np.pad(features, ((0, 0), (0, 0), (R, R), (R, R)))  # [N,C,H+4,W+4]
    wl_idx = np.arange(L)
    for core in range(8):
        n, q = divmod(core, 4)
        h0 = HC * q
        F_core = padded[n, :, h0 : h0 + NR, :]  # [C, 29, 104]
        F_host = np.zeros((KB, NBLK, NR, C), np.float16)
        for blk in range(NBLK):
            F_host[:, blk] = F_core[:, :, L * blk : L * blk + KB].transpose(2, 1, 0)
        # masks[n]: [25, 200, 200] -> [dy, dx, h, a, w, b]
        m7 = masks[n].reshape(KS, KS, H, S, W, S)[:, :, h0 : h0 + HC]
        B_host = np.zeros((KB, NBLK, HC, KS, 2, L, 2), np.float16)
        for blk in range(NBLK):
            for dx in range(KS):
                src = m7[:, dx, :, :, L * blk : L * blk + L, :]  # [dy,h,a,wl,b]
                B_host[dx + wl_idx, blk, :, :, :, wl_idx, :] = (
                    src.transpose(3, 1, 0, 2, 4)
                )
        f_hosts.append(F_host)
        b_hosts.append(B_host.reshape(KB, NBLK, HC, KS, NCOL))
    return f_hosts, b_hosts


# ---------------- v2: 128-partition layout, per-block PSUM banks ----------------
KB2 = 64  # padded contraction size (54 useful + 10 zero rows) -> blocks at 0/64


def build_program_v2(iters: int = 1, dt=F16, copy_eng="both", parts="full",
                     psbufs: int = 3, obufs: int = 2, out_group: int = 5,
                     bchunks: int = 5):
    """v2: both width blocks packed on 128 partitions (bases 0/64), each block
    accumulating into its own PSUM bank (documented-safe row-tiling pattern).
    dy-inner loop: weights reload per matmul but the two block chains run
    concurrently on different PE row groups."""
    nc = bacc.Bacc(None, target_bir_lowering=False, debug=False)
    f_in = nc.dram_tensor("f", [128, NR, C], dt, kind="ExternalInput")
    b_in = nc.dram_tensor("b", [128, HC, KS, NCOL], dt, kind="ExternalInput")
    out = nc.dram_tensor("out", [C, S * HC, S * W], F32, kind="ExternalOutput")

    with tile.TileContext(nc) as tc:
        with (
            tc.tile_pool(name="fsb", bufs=1) as fpool,
            tc.tile_pool(name="bsb", bufs=1) as bpool,
            tc.tile_pool(name="osb", bufs=obufs) as opool,
            tc.tile_pool(name="ps0", bufs=psbufs, space="PSUM") as pspool0,
            tc.tile_pool(name="ps1", bufs=psbufs, space="PSUM") as pspool1,
        ):
            pspools = [pspool0, pspool1]

            def body(_=None):
                F_sb = fpool.tile([128, NR, C], dt)
                B_sb = bpool.tile([128, HC, KS, NCOL], dt)
                if parts != "nodmain":
                    # chunked input DMAs: lets matmuls start after chunk 0
                    nc.sync.dma_start(F_sb[:, : NR // 2], f_in[:, : NR // 2])
                    nc.sync.dma_start(F_sb[:, NR // 2 :], f_in[:, NR // 2 :])
                    bstep = (HC + bchunks - 1) // bchunks
                    for h0 in range(0, HC, bstep):
                        h1 = min(h0 + bstep, HC)
                        nc.sync.dma_start(B_sb[:, h0:h1], b_in[:, h0:h1])
                if parts == "dmain":
                    return
                G = out_group
                for ct in range(2):
                    for g0 in range(0, HC, G):
                        g1 = min(g0 + G, HC)
                        osb = opool.tile([128, G, 2, NBLK * S * L], F32)
                        for h in range(g0, g1):
                            ps = [
                                pspools[blk].tile(
                                    [128, NCOL], F32, name=f"ps{blk}_{ct}_{h}",
                                    tag=f"psb{blk}",
                                )
                                for blk in range(NBLK)
                            ]
                            for dy in range(KS):
                                for blk in range(NBLK):
                                    lo = KB2 * blk
                                    nc.tensor.matmul(
                                        ps[blk][:, :],
                                        F_sb[lo : lo + KB2, h + dy,
                                             ct * 128 : (ct + 1) * 128],
                                        B_sb[lo : lo + KB2, h, dy, :],
                                        start=(dy == 0),
                                        stop=(dy == KS - 1),
                                    )
                            if parts == "nocopy":
                                continue
                            # osb free layout per h: (a, blk, w2l) built from the
                            # two psum tiles; dest dims [2, (blk, 100)]
                            dstv = osb[:, h - g0].rearrange(
                                "p a (k w) -> p a k w", k=NBLK
                            )
                            for blk in range(NBLK):
                                src = ps[blk][:].rearrange("p (a w) -> p a w", a=2)
                                dst = dstv[:, :, blk, :]
                                if copy_eng == "vector" or (
                                    copy_eng == "both" and blk == 0
                                ):
                                    nc.vector.tensor_copy(dst, src)
                                else:
                                    nc.scalar.copy(dst, src)
                        if parts == "nocopy":
                            continue
                        nc.sync.dma_start(
                            out[ct * 128 : (ct + 1) * 128,
                                S * g0 : S * g1, :],
                            osb[:, : g1 - g0].rearrange("p g a c -> p (g a c)"),
                        )

            if iters == 1:
                body()
            else:
                with tc.For_i(0, iters, 1) as _i:
                    body(_i)
    nc.finalize()
    return nc


def host_prep_v2(features: np.ndarray, masks: np.ndarray):
    """v2 layouts: [128, ...] with partition = 64*blk + w'' (w'' in [0,54))."""
    f_hosts, b_hosts = [], []
    padded = np.pad(features, ((0, 0), (0, 0), (R, R), (R, R)))
    wl_idx = np.arange(L)
    for core in range(8):
        n, q = divmod(core, 4)
        h0 = HC * q
        F_core = padded[n, :, h0 : h0 + NR, :]  # [C, 29, 104]
        F_host = np.zeros((128, NR, C), np.float16)
        for blk in range(NBLK):
            F_host[KB2 * blk : KB2 * blk + KB] = (
                F_core[:, :, L * blk : L * blk + KB].transpose(2, 1, 0)
            )
        m7 = masks[n].reshape(KS, KS, H, S, W, S)[:, :, h0 : h0 + HC]
        B_host = np.zeros((128, HC, KS, 2, L, 2), np.float16)
        for blk in range(NBLK):
            for dx in range(KS):
                src = m7[:, dx, :, :, L * blk : L * blk + L, :]  # [dy,h,a,wl,b]
                B_host[KB2 * blk + dx + wl_idx, :, :, :, wl_idx, :] = (
                    src.transpose(3, 1, 0, 2, 4)
                )
        f_hosts.append(F_host)
        b_hosts.append(B_host.reshape(128, HC, KS, NCOL))
    return f_hosts, b_hosts


# ---------------- v3: dy-pairs stacked in K (two taps per matmul) ----------------
NP3 = (KS + 1) // 2  # 3 matmuls per (h, blk): dy pairs (0,1), (2,3), (4,-)


def build_program_v3(iters: int = 1, dt=F16, copy_eng="both", parts="full",
                     psbufs: int = 3, obufs: int = 2, out_group: int = 5,
                     bchunks: int = 5, unroll: bool = False):
    """v3: K=128 = (dy-pair half j in {0,1}) x (w'' in [0,64)). The upper 64
    partitions hold a one-row-shifted copy of the features, so one matmul
    contracts two vertical taps. 300 matmuls of N=200, all tile_position
    (0,0), one PSUM bank per output row."""
    nc = bacc.Bacc(None, target_bir_lowering=False, debug=False)
    f_in = nc.dram_tensor("f", [128, NBLK, NR, C], dt, kind="ExternalInput")
    b_in = nc.dram_tensor("b", [128, NBLK, HC, NP3, NCOL], dt, kind="ExternalInput")
    out = nc.dram_tensor("out", [C, S * HC, S * W], F32, kind="ExternalOutput")

    with tile.TileContext(nc) as tc:
        with (
            tc.tile_pool(name="fsb", bufs=1) as fpool,
            tc.tile_pool(name="bsb", bufs=1) as bpool,
            tc.tile_pool(name="osb", bufs=obufs) as opool,
            tc.tile_pool(name="ps", bufs=psbufs, space="PSUM") as pspool,
        ):
            def body(_=None):
                F_sb = fpool.tile([128, NBLK, NR, C], dt)
                B_sb = bpool.tile([128, NBLK, HC, NP3, NCOL], dt)
                if parts != "nodmain":
                    nc.sync.dma_start(F_sb[:, :, : NR // 2], f_in[:, :, : NR // 2])
                    nc.sync.dma_start(F_sb[:, :, NR // 2 :], f_in[:, :, NR // 2 :])
                    bstep = (HC + bchunks - 1) // bchunks
                    for h0 in range(0, HC, bstep):
                        h1 = min(h0 + bstep, HC)
                        nc.sync.dma_start(B_sb[:, :, h0:h1], b_in[:, :, h0:h1])
                if parts == "dmain":
                    return
                G = out_group
                for ct in range(2):
                    for g0 in range(0, HC, G):
                        g1 = min(g0 + G, HC)
                        osb = opool.tile([128, G, 2, NBLK * S * L], F32)
                        for h in range(g0, g1):
                            ps = pspool.tile(
                                [128, NBLK * NCOL], F32, name=f"ps_{ct}_{h}",
                                tag="ps",
                            )
                            for blk in range(NBLK):
                                for p in range(NP3):
                                    nc.tensor.matmul(
                                        ps[:, blk * NCOL : (blk + 1) * NCOL],
                                        F_sb[:, blk, h + 2 * p,
                                             ct * 128 : (ct + 1) * 128],
                                        B_sb[:, blk, h, p, :],
                                        start=(blk == 0 and p == 0),
                                        stop=(blk == NBLK - 1 and p == NP3 - 1),
                                    )
                            if parts == "nocopy":
                                continue
                            # psum free layout (blk, a, w2l) -> dest (a, blk, w2l)
                            src = ps[:].rearrange("p (k a w) -> p a k w", k=NBLK, a=2)
                            dst = osb[:, h - g0].rearrange(
                                "p a (k w) -> p a k w", k=NBLK
                            )
                            if copy_eng == "vector" or (
                                copy_eng == "both" and h % 2 == 0
                            ):
                                nc.vector.tensor_copy(dst, src)
                            else:
                                nc.scalar.copy(dst, src)
                        if parts == "nocopy":
                            continue
                        nc.sync.dma_start(
                            out[ct * 128 : (ct + 1) * 128, S * g0 : S * g1, :],
                            osb[:, : g1 - g0].rearrange("p g a c -> p (g a c)"),
                        )

            if iters == 1:
                body()
            elif unroll:
                for _k in range(iters):
                    body(_k)
            else:
                with tc.For_i(0, iters, 1) as _i:
                    body(_i)
    nc.finalize()
    return nc


def host_prep_v3(features: np.ndarray, masks: np.ndarray):
    """v3 layouts: partition = 64*j + w''; j=1 half holds features shifted one
    row down (dy-pair trick). Separate windows per width block."""
    f_hosts, b_hosts = [], []
    padded = np.pad(features, ((0, 0), (0, 0), (R, R), (R, R)))
    wl_idx = np.arange(L)
    for core in range(8):
        n, q = divmod(core, 4)
        h0 = HC * q
        F_core = padded[n, :, h0 : h0 + NR, :]  # [C, 29, 104]
        F_host = np.zeros((128, NBLK, NR, C), np.float16)
        for blk in range(NBLK):
            win = F_core[:, :, L * blk : L * blk + KB].transpose(2, 1, 0)  # [54,29,C]
            F_host[:KB, blk] = win                      # j=0: rows r
            F_host[64 : 64 + KB, blk, : NR - 1] = win[:, 1:]  # j=1: rows r+1
        m7 = masks[n].reshape(KS, KS, H, S, W, S)[:, :, h0 : h0 + HC]
        B_host = np.zeros((128, NBLK, HC, NP3, 2, L, 2), np.float16)
        for blk in range(NBLK):
            for dx in range(KS):
                for dy in range(KS):
                    p, j = divmod(dy, 2)
                    src = m7[dy, dx, :, :, L * blk : L * blk + L, :]  # [h,a,wl,b]
                    B_host[64 * j + dx + wl_idx, blk, :, p, :, wl_idx, :] = (
                        src.transpose(2, 0, 1, 3)
                    )
        f_hosts.append(F_host)
        b_hosts.append(B_host.reshape(128, NBLK, HC, NP3, NCOL))
    return f_hosts, b_hosts


# ---------------- v5: 2D-banded tiles, all 25 taps in one K=128 matmul ----------------
# Output tiled into (DT=5 low-res rows) x (LW=10 low-res cols) tiles; per tile the
# contraction dim packs the full 2D source window: K = (DT+4 rows) x (LW+4 cols)
# = 9*14 = 126 (pad 128). One matmul per (ct, t, u): N = DT*2*LW*2 = 200 output
# pixels, no PSUM accumulation chains. 100 matmuls/core instead of 500, input
# traffic 5.8 MB (vs 8.3), output fp16 (5.1 MB vs 10.2 f32; host casts back).
DT5 = 5            # low-res rows per tile
LW5 = 10           # low-res cols per tile
NT5 = HC // DT5    # 5 h-tiles per core
NU5 = W // LW5     # 10 w-tiles
KR5 = DT5 + KS - 1   # 9 source rows per tile
KW5 = LW5 + KS - 1   # 14 source cols per tile (K = 9*14 = 126)
NC5 = DT5 * S * LW5 * S  # 200 output pixels per tile
NTU5 = NT5 * NU5   # 50 tiles


def build_program_v5(iters: int = 1, dt=F16, odt=F16, chunks: int = 5,
                     obufs: int = 3, psbufs: int = 8, parts: str = "full"):
    nc = bacc.Bacc(None, target_bir_lowering=False, debug=False)
    f_in = nc.dram_tensor("f", [128, NTU5, C], dt, kind="ExternalInput")
    b_in = nc.dram_tensor("b", [128, NTU5, NC5], dt, kind="ExternalInput")
    out = nc.dram_tensor("out", [C, S * HC, S * W], odt, kind="ExternalOutput")

    with tile.TileContext(nc) as tc:
        with (
            tc.tile_pool(name="fsb", bufs=1) as fpool,
            tc.tile_pool(name="bsb", bufs=1) as bpool,
            tc.tile_pool(name="osb", bufs=obufs) as opool,
            tc.tile_pool(name="ps", bufs=psbufs, space="PSUM") as pspool,
        ):
            def body(_=None):
                F_sb = fpool.tile([128, NTU5, C], dt)
                B_sb = bpool.tile([128, NTU5, NC5], dt)
                if parts != "nodmain":
                    step = (NTU5 + chunks - 1) // chunks
                    for c0 in range(0, NTU5, step):
                        c1 = min(c0 + step, NTU5)
                        nc.sync.dma_start(F_sb[:, c0:c1], f_in[:, c0:c1])
                        nc.sync.dma_start(B_sb[:, c0:c1], b_in[:, c0:c1])
                if parts == "dmain":
                    return
                ei = 0
                for ct in range(2):
                    for t in range(NT5):
                        osb = opool.tile([128, S * DT5, S * W], odt)
                        for u in range(NU5):
                            tu = t * NU5 + u
                            ps = pspool.tile([128, NC5], F32,
                                             name=f"ps_{ct}_{t}_{u}", tag="ps")
                            nc.tensor.matmul(
                                ps[:],
                                F_sb[:, tu, ct * 128 : (ct + 1) * 128],
                                B_sb[:, tu, :],
                                start=True,
                                stop=True,
                            )
                            if parts == "nocopy":
                                continue
                            # psum (h', a, w, b) -> osb rows (h', a), cols (u, w, b)
                            src = ps[:].rearrange(
                                "p (h a w b) -> p (h a) (w b)", h=DT5, a=2, w=LW5
                            )
                            dst = osb[:, :, S * LW5 * u : S * LW5 * (u + 1)]
                            if ei % 2 == 0:
                                nc.vector.tensor_copy(dst, src)
                            else:
                                nc.scalar.copy(dst, src)
                            ei += 1
                        if parts == "nocopy":
                            continue
                        nc.sync.dma_start(
                            out[ct * 128 : (ct + 1) * 128,
                                S * DT5 * t : S * DT5 * (t + 1), :],
                            osb[:],
                        )

            if iters == 1:
                body()
            else:
                with tc.For_i(0, iters, 1) as _i:
                    body(_i)
    nc.finalize()
    return nc


def host_prep_v5(features: np.ndarray, masks: np.ndarray):
    """v5 layouts. Per core (n, q):
    f: [128, 50, 256]  f[p=(rho*14+wp), (t,u), c] = fpad[c, 25q+5t+rho, 10u+wp]
    b: [128, 50, 200]  b[p, (t,u), (h',a,w,b)] = m[dy=rho-h', dx=wp-w, ...] (2D band)
    """
    padded = np.pad(features, ((0, 0), (0, 0), (R, R), (R, R))).astype(np.float16)
    hh = np.repeat(np.arange(DT5), LW5)   # h' per (h', w) pair
    ww = np.tile(np.arange(LW5), DT5)     # w
    f_hosts, b_hosts = [], []
    for core in range(8):
        n, q = divmod(core, 4)
        h0 = HC * q
        pn = padded[n]  # [C, 104, 104]
        F = np.zeros((128, NT5, NU5, C), np.float16)
        for rho in range(KR5):
            for wp in range(KW5):
                sl = pn[:, h0 + rho : h0 + rho + HC : DT5, wp : wp + W : LW5]
                F[rho * KW5 + wp] = sl.transpose(1, 2, 0)
        # masks[n]: [25, 200, 200] -> [dy, dx, h, a, w, b]
        m6 = masks[n].reshape(KS, KS, H, S, W, S)[:, :, h0 : h0 + HC]
        m6 = m6.astype(np.float16)
        B = np.zeros((128, NT5, NU5, DT5, 2, LW5, 2), np.float16)
        for dy in range(KS):
            for dx in range(KS):
                pp = (hh + dy) * KW5 + (ww + dx)
                # src [t, h', a, u, w, b]
                src = m6[dy, dx].reshape(NT5, DT5, 2, NU5, LW5, 2)
                # fancy: result dims [pair, t, u, a, b]
                B[pp, :, :, hh, :, ww, :] = src[:, hh, :, :, ww, :].transpose(
                    0, 1, 3, 2, 4
                )
        f_hosts.append(F.reshape(128, NTU5, C))
        b_hosts.append(B.reshape(128, NTU5, NC5))
    return f_hosts, b_hosts


_NC_CACHE = {}

BUILD = build_program_v5
PREP = host_prep_v5


def _get_program(iters: int = 1):
    # v5: 2D-banded single-matmul-per-tile formulation (see build_program_v5).
    if iters not in _NC_CACHE:
        _NC_CACHE[iters] = BUILD(iters)
    return _NC_CACHE[iters]


def kernel(features: np.ndarray, masks: np.ndarray) -> np.ndarray:
    features = np.ascontiguousarray(features, dtype=np.float32)
    masks = np.ascontiguousarray(masks, dtype=np.float32)
    f_hosts, b_hosts = PREP(features, masks)
    in_maps = [{"f": f_hosts[c], "b": b_hosts[c]} for c in range(8)]
    nc = _get_program(1)
    res = run_bass_kernel_spmd(nc, in_maps, list(range(8)))
    out = np.empty((N, C, S * H, S * W), np.float32)
    for core in range(8):
        n, q = divmod(core, 4)
        out[n, :, S * HC * q : S * HC * (q + 1), :] = res.results[core][
            "out"
        ].astype(np.float32)
    return out



# revision 17
# speedup vs baseline: 2.2361x; 2.2361x over previous
"""CARAFE-naive 2x content-aware upsampling on 8 Trainium2 NeuronCores.

Problem: features [2, 256, 100, 100] f32, masks [2, 25, 200, 200] f32
-> out [2, 256, 200, 200] f32, where each output pixel is a 25-tap (5x5)
weighted sum of the source neighborhood, weights shared across channels.

Strategy (per core = one (image n, row-quarter q) pair) -- v5, 2D band:
  The output is tiled into (5 low-res rows x 10 low-res cols) tiles; for
  each tile ALL 25 taps are contracted in a single TensorE matmul with
  K = (5+4 rows) x (10+4 cols) = 126 source-window positions (padded to
  128): out[c, pix] = sum_{rho,w'} F[(rho,w'), c] * B[(rho,w'), pix],
  where B is the mask tensor banded in both spatial dims (nonzero only
  where (rho-h', w'-w) is a valid tap). 100 matmuls/core (N=200, full
  K=128, no PSUM accumulation chains), 2 per PSUM bank, then contiguous
  f32->fp16 copies (vector/scalar alternating) and fp16 output DMAs in
  raw PSUM order; the host undoes the pixel permutation (post_v5).

  Input DMAs stream on the sync engine's HWDGE ring (small first chunk
  so matmuls start early), output DMAs on the scalar engine's ring so
  the two directions overlap. Both operands fp16 (PE full rate).

Host-side numpy does layout/packing only (pad, strided window gather,
2D-banded mask scatter, output permutation); all FLOPs on the device.
Older v1/v2/v3 single-axis-band variants kept below for reference.
"""

import numpy as np

import concourse.mybir as mybir
import concourse.tile as tile
from concourse import bacc
from concourse.bass_utils import run_bass_kernel_spmd

# problem constants
N, C, H, W = 2, 256, 100, 100
KS = 5        # kernel size
S = 2         # upsample scale
R = (KS - 1) // 2

# sharding / blocking constants
HC = H // 4       # 25 low-res rows per core (8 cores = 2 images x 4 quarters)
NR = HC + 2 * R   # 29 padded feature rows per core
NBLK = 2          # width blocks
L = W // NBLK     # 50 low-res columns per block
KB = L + KS - 1   # 54 = matmul contraction size
PBASE = 64        # SBUF partition base stride between blocks
NCOL = 2 * S * L  # 200 matmul N per block: (a in 2, w2l in 100)
F16 = mybir.dt.float16
F32 = mybir.dt.float32


def build_program(iters: int = 1, dt=F16, blks=(0, 1), copy_eng="both", parts="full",
                  in_chunks: int = 1, in_engines=("sync",)):
    """Build the per-core bass program. `iters`>1 wraps the whole compute in
    a hardware loop (used only for benchmarking slope timing)."""
    nc = bacc.Bacc(None, target_bir_lowering=False, debug=False)
    f_in = nc.dram_tensor("f", [KB, NBLK, NR, C], dt, kind="ExternalInput")
    b_in = nc.dram_tensor("b", [KB, NBLK, HC, KS, NCOL], dt, kind="ExternalInput")
    out = nc.dram_tensor("out", [C, S * HC, S * W], F32, kind="ExternalOutput")

    with tile.TileContext(nc) as tc:
        with (
            tc.tile_pool(name="fsb", bufs=1) as fpool,
            tc.tile_pool(name="bsb", bufs=1) as bpool,
            tc.tile_pool(name="osb", bufs=4) as opool,
            tc.tile_pool(name="ps", bufs=6, space="PSUM") as pspool,
        ):
            def body(_=None):
                F_sb = fpool.tile([KB, NBLK, NR, C], dt)
                B_sb = bpool.tile([KB, NBLK, HC, KS, NCOL], dt)
                if parts == "dmain128":
                    # DMA-bandwidth probe: same bytes, 108-partition layout
                    F2 = fpool.tile([KB * NBLK, NR, C], dt, name="F2")
                    B2 = bpool.tile([KB * NBLK, HC, KS, NCOL], dt, name="B2")
                    f2 = f_in[:].rearrange("k n r c -> (k n) r c")
                    b2 = b_in[:].rearrange("k n h d c -> (k n) h d c")
                    engs = [getattr(nc, e) for e in in_engines]
                    step = (KB * NBLK + in_chunks - 1) // in_chunks
                    for i, p0 in enumerate(range(0, KB * NBLK, step)):
                        p1 = min(p0 + step, KB * NBLK)
                        engs[i % len(engs)].dma_start(F2[p0:p1], f2[p0:p1])
                        engs[i % len(engs)].dma_start(B2[p0:p1], b2[p0:p1])
                    return
                if parts != "nodmain":
                    engs = [getattr(nc, e) for e in in_engines]
                    ei = 0
                    # split each input DMA into in_chunks along a free dim to
                    # engage more DMA queues in parallel
                    fstep = (NR + in_chunks - 1) // in_chunks
                    for r0 in range(0, NR, fstep):
                        r1 = min(r0 + fstep, NR)
                        engs[ei % len(engs)].dma_start(
                            F_sb[:, :, r0:r1], f_in[:, :, r0:r1]
                        )
                        ei += 1
                    bstep = (HC + in_chunks - 1) // in_chunks
                    for h0 in range(0, HC, bstep):
                        h1 = min(h0 + bstep, HC)
                        engs[ei % len(engs)].dma_start(
                            B_sb[:, :, h0:h1], b_in[:, :, h0:h1]
                        )
                        ei += 1
                if parts == "dmain":
                    return
                for ct in range(2):
                    psums = {}
                    for r in range(NR):
                        for blk in blks:
                            lhsT = F_sb[:, blk, r, ct * 128 : (ct + 1) * 128]
                            for dy in range(KS):
                                h = r - dy
                                if not (0 <= h < HC):
                                    continue
                                if dy == 0 and blk == blks[0]:
                                    psums[h] = pspool.tile(
                                        [128, NBLK * NCOL],
                                        F32,
                                        name=f"ps{ct}_{h}",
                                        tag="ps",
                                    )
                                # One accumulation group per PSUM bank: start
                                # zeroes the whole 2KB zero-region, so only
                                # the first matmul of the tile starts and only
                                # the last one stops.
                                nc.tensor.matmul(
                                    psums[h][:, blk * NCOL : (blk + 1) * NCOL],
                                    lhsT,
                                    B_sb[:, blk, h, dy, :],
                                    start=(dy == 0 and blk == blks[0]),
                                    stop=(dy == KS - 1 and blk == blks[-1]),
                                )
                        h_done = r - (KS - 1)
                        if h_done >= 0 and parts in ("full", "nodmain"):
                            ps = psums.pop(h_done)
                            osb = opool.tile([128, 2, NBLK, S * L], F32)
                            # psum free layout (blk, a, w2l) -> (a, blk, w2l)
                            src = ps[:].rearrange(
                                "p (k a w) -> p a k w", k=NBLK, a=2
                            )
                            if copy_eng == "vector" or (copy_eng == "both" and h_done % 2 == 0):
                                nc.vector.tensor_copy(osb[:], src)
                            else:
                                nc.scalar.copy(osb[:], src)
                            nc.sync.dma_start(
                                out[ct * 128 : (ct + 1) * 128,
                                    S * h_done : S * h_done + 2, :],
                                osb[:].rearrange("p a k w -> p a (k w)"),
                            )

            if iters == 1:
                body()
            else:
                with tc.For_i(0, iters, 1) as _i:
                    body(_i)
    nc.finalize()
    return nc


def host_prep(features: np.ndarray, masks: np.ndarray):
    """Pack per-core fp16 inputs: transposed padded feature rows and banded
    mask matrices. Pure layout work (no arithmetic beyond dtype cast)."""
    f_hosts, b_hosts = [], []
    padded = np.pad(features, ((0, 0), (0, 0), (R, R), (R, R)))  # [N,C,H+4,W+4]
    wl_idx = np.arange(L)
    for core in range(8):
        n, q = divmod(core, 4)
        h0 = HC * q
        F_core = padded[n, :, h0 : h0 + NR, :]  # [C, 29, 104]
        F_host = np.zeros((KB, NBLK, NR, C), np.float16)
        for blk in range(NBLK):
            F_host[:, blk] = F_core[:, :, L * blk : L * blk + KB].transpose(2, 1, 0)
        # masks[n]: [25, 200, 200] -> [dy, dx, h, a, w, b]
        m7 = masks[n].reshape(KS, KS, H, S, W, S)[:, :, h0 : h0 + HC]
        B_host = np.zeros((KB, NBLK, HC, KS, 2, L, 2), np.float16)
        for blk in range(NBLK):
            for dx in range(KS):
                src = m7[:, dx, :, :, L * blk : L * blk + L, :]  # [dy,h,a,wl,b]
                B_host[dx + wl_idx, blk, :, :, :, wl_idx, :] = (
                    src.transpose(3, 1, 0, 2, 4)
                )
        f_hosts.append(F_host)
        b_hosts.append(B_host.reshape(KB, NBLK, HC, KS, NCOL))
    return f_hosts, b_hosts


# ---------------- v2: 128-partition layout, per-block PSUM banks ----------------
KB2 = 64  # padded contraction size (54 useful + 10 zero rows) -> blocks at 0/64


def build_program_v2(iters: int = 1, dt=F16, copy_eng="both", parts="full",
                     psbufs: int = 3, obufs: int = 2, out_group: int = 5,
                     bchunks: int = 5):
    """v2: both width blocks packed on 128 partitions (bases 0/64), each block
    accumulating into its own PSUM bank (documented-safe row-tiling pattern).
    dy-inner loop: weights reload per matmul but the two block chains run
    concurrently on different PE row groups."""
    nc = bacc.Bacc(None, target_bir_lowering=False, debug=False)
    f_in = nc.dram_tensor("f", [128, NR, C], dt, kind="ExternalInput")
    b_in = nc.dram_tensor("b", [128, HC, KS, NCOL], dt, kind="ExternalInput")
    out = nc.dram_tensor("out", [C, S * HC, S * W], F32, kind="ExternalOutput")

    with tile.TileContext(nc) as tc:
        with (
            tc.tile_pool(name="fsb", bufs=1) as fpool,
            tc.tile_pool(name="bsb", bufs=1) as bpool,
            tc.tile_pool(name="osb", bufs=obufs) as opool,
            tc.tile_pool(name="ps0", bufs=psbufs, space="PSUM") as pspool0,
            tc.tile_pool(name="ps1", bufs=psbufs, space="PSUM") as pspool1,
        ):
            pspools = [pspool0, pspool1]

            def body(_=None):
                F_sb = fpool.tile([128, NR, C], dt)
                B_sb = bpool.tile([128, HC, KS, NCOL], dt)
                if parts != "nodmain":
                    # chunked input DMAs: lets matmuls start after chunk 0
                    nc.sync.dma_start(F_sb[:, : NR // 2], f_in[:, : NR // 2])
                    nc.sync.dma_start(F_sb[:, NR // 2 :], f_in[:, NR // 2 :])
                    bstep = (HC + bchunks - 1) // bchunks
                    for h0 in range(0, HC, bstep):
                        h1 = min(h0 + bstep, HC)
                        nc.sync.dma_start(B_sb[:, h0:h1], b_in[:, h0:h1])
                if parts == "dmain":
                    return
                G = out_group
                for ct in range(2):
                    for g0 in range(0, HC, G):
                        g1 = min(g0 + G, HC)
                        osb = opool.tile([128, G, 2, NBLK * S * L], F32)
                        for h in range(g0, g1):
                            ps = [
                                pspools[blk].tile(
                                    [128, NCOL], F32, name=f"ps{blk}_{ct}_{h}",
                                    tag=f"psb{blk}",
                                )
                                for blk in range(NBLK)
                            ]
                            for dy in range(KS):
                                for blk in range(NBLK):
                                    lo = KB2 * blk
                                    nc.tensor.matmul(
                                        ps[blk][:, :],
                                        F_sb[lo : lo + KB2, h + dy,
                                             ct * 128 : (ct + 1) * 128],
                                        B_sb[lo : lo + KB2, h, dy, :],
                                        start=(dy == 0),
                                        stop=(dy == KS - 1),
                                    )
                            if parts == "nocopy":
                                continue
                            # osb free layout per h: (a, blk, w2l) built from the
                            # two psum tiles; dest dims [2, (blk, 100)]
                            dstv = osb[:, h - g0].rearrange(
                                "p a (k w) -> p a k w", k=NBLK
                            )
                            for blk in range(NBLK):
                                src = ps[blk][:].rearrange("p (a w) -> p a w", a=2)
                                dst = dstv[:, :, blk, :]
                                if copy_eng == "vector" or (
                                    copy_eng == "both" and blk == 0
                                ):
                                    nc.vector.tensor_copy(dst, src)
                                else:
                                    nc.scalar.copy(dst, src)
                        if parts == "nocopy":
                            continue
                        nc.sync.dma_start(
                            out[ct * 128 : (ct + 1) * 128,
                                S * g0 : S * g1, :],
                            osb[:, : g1 - g0].rearrange("p g a c -> p (g a c)"),
                        )

            if iters == 1:
                body()
            else:
                with tc.For_i(0, iters, 1) as _i:
                    body(_i)
    nc.finalize()
    return nc


def host_prep_v2(features: np.ndarray, masks: np.ndarray):
    """v2 layouts: [128, ...] with partition = 64*blk + w'' (w'' in [0,54))."""
    f_hosts, b_hosts = [], []
    padded = np.pad(features, ((0, 0), (0, 0), (R, R), (R, R)))
    wl_idx = np.arange(L)
    for core in range(8):
        n, q = divmod(core, 4)
        h0 = HC * q
        F_core = padded[n, :, h0 : h0 + NR, :]  # [C, 29, 104]
        F_host = np.zeros((128, NR, C), np.float16)
        for blk in range(NBLK):
            F_host[KB2 * blk : KB2 * blk + KB] = (
                F_core[:, :, L * blk : L * blk + KB].transpose(2, 1, 0)
            )
        m7 = masks[n].reshape(KS, KS, H, S, W, S)[:, :, h0 : h0 + HC]
        B_host = np.zeros((128, HC, KS, 2, L, 2), np.float16)
        for blk in range(NBLK):
            for dx in range(KS):
                src = m7[:, dx, :, :, L * blk : L * blk + L, :]  # [dy,h,a,wl,b]
                B_host[KB2 * blk + dx + wl_idx, :, :, :, wl_idx, :] = (
                    src.transpose(3, 1, 0, 2, 4)
                )
        f_hosts.append(F_host)
        b_hosts.append(B_host.reshape(128, HC, KS, NCOL))
    return f_hosts, b_hosts


# ---------------- v3: dy-pairs stacked in K (two taps per matmul) ----------------
NP3 = (KS + 1) // 2  # 3 matmuls per (h, blk): dy pairs (0,1), (2,3), (4,-)


def build_program_v3(iters: int = 1, dt=F16, copy_eng="both", parts="full",
                     psbufs: int = 3, obufs: int = 2, out_group: int = 5,
                     bchunks: int = 5, unroll: bool = False):
    """v3: K=128 = (dy-pair half j in {0,1}) x (w'' in [0,64)). The upper 64
    partitions hold a one-row-shifted copy of the features, so one matmul
    contracts two vertical taps. 300 matmuls of N=200, all tile_position
    (0,0), one PSUM bank per output row."""
    nc = bacc.Bacc(None, target_bir_lowering=False, debug=False)
    f_in = nc.dram_tensor("f", [128, NBLK, NR, C], dt, kind="ExternalInput")
    b_in = nc.dram_tensor("b", [128, NBLK, HC, NP3, NCOL], dt, kind="ExternalInput")
    out = nc.dram_tensor("out", [C, S * HC, S * W], F32, kind="ExternalOutput")

    with tile.TileContext(nc) as tc:
        with (
            tc.tile_pool(name="fsb", bufs=1) as fpool,
            tc.tile_pool(name="bsb", bufs=1) as bpool,
            tc.tile_pool(name="osb", bufs=obufs) as opool,
            tc.tile_pool(name="ps", bufs=psbufs, space="PSUM") as pspool,
        ):
            def body(_=None):
                F_sb = fpool.tile([128, NBLK, NR, C], dt)
                B_sb = bpool.tile([128, NBLK, HC, NP3, NCOL], dt)
                if parts != "nodmain":
                    nc.sync.dma_start(F_sb[:, :, : NR // 2], f_in[:, :, : NR // 2])
                    nc.sync.dma_start(F_sb[:, :, NR // 2 :], f_in[:, :, NR // 2 :])
                    bstep = (HC + bchunks - 1) // bchunks
                    for h0 in range(0, HC, bstep):
                        h1 = min(h0 + bstep, HC)
                        nc.sync.dma_start(B_sb[:, :, h0:h1], b_in[:, :, h0:h1])
                if parts == "dmain":
                    return
                G = out_group
                for ct in range(2):
                    for g0 in range(0, HC, G):
                        g1 = min(g0 + G, HC)
                        osb = opool.tile([128, G, 2, NBLK * S * L], F32)
                        for h in range(g0, g1):
                            ps = pspool.tile(
                                [128, NBLK * NCOL], F32, name=f"ps_{ct}_{h}",
                                tag="ps",
                            )
                            for blk in range(NBLK):
                                for p in range(NP3):
                                    nc.tensor.matmul(
                                        ps[:, blk * NCOL : (blk + 1) * NCOL],
                                        F_sb[:, blk, h + 2 * p,
                                             ct * 128 : (ct + 1) * 128],
                                        B_sb[:, blk, h, p, :],
                                        start=(blk == 0 and p == 0),
                                        stop=(blk == NBLK - 1 and p == NP3 - 1),
                                    )
                            if parts == "nocopy":
                                continue
                            # psum free layout (blk, a, w2l) -> dest (a, blk, w2l)
                            src = ps[:].rearrange("p (k a w) -> p a k w", k=NBLK, a=2)
                            dst = osb[:, h - g0].rearrange(
                                "p a (k w) -> p a k w", k=NBLK
                            )
                            if copy_eng == "vector" or (
                                copy_eng == "both" and h % 2 == 0
                            ):
                                nc.vector.tensor_copy(dst, src)
                            else:
                                nc.scalar.copy(dst, src)
                        if parts == "nocopy":
                            continue
                        nc.sync.dma_start(
                            out[ct * 128 : (ct + 1) * 128, S * g0 : S * g1, :],
                            osb[:, : g1 - g0].rearrange("p g a c -> p (g a c)"),
                        )

            if iters == 1:
                body()
            elif unroll:
                for _k in range(iters):
                    body(_k)
            else:
                with tc.For_i(0, iters, 1) as _i:
                    body(_i)
    nc.finalize()
    return nc


def host_prep_v3(features: np.ndarray, masks: np.ndarray):
    """v3 layouts: partition = 64*j + w''; j=1 half holds features shifted one
    row down (dy-pair trick). Separate windows per width block."""
    f_hosts, b_hosts = [], []
    padded = np.pad(features, ((0, 0), (0, 0), (R, R), (R, R)))
    wl_idx = np.arange(L)
    for core in range(8):
        n, q = divmod(core, 4)
        h0 = HC * q
        F_core = padded[n, :, h0 : h0 + NR, :]  # [C, 29, 104]
        F_host = np.zeros((128, NBLK, NR, C), np.float16)
        for blk in range(NBLK):
            win = F_core[:, :, L * blk : L * blk + KB].transpose(2, 1, 0)  # [54,29,C]
            F_host[:KB, blk] = win                      # j=0: rows r
            F_host[64 : 64 + KB, blk, : NR - 1] = win[:, 1:]  # j=1: rows r+1
        m7 = masks[n].reshape(KS, KS, H, S, W, S)[:, :, h0 : h0 + HC]
        B_host = np.zeros((128, NBLK, HC, NP3, 2, L, 2), np.float16)
        for blk in range(NBLK):
            for dx in range(KS):
                for dy in range(KS):
                    p, j = divmod(dy, 2)
                    src = m7[dy, dx, :, :, L * blk : L * blk + L, :]  # [h,a,wl,b]
                    B_host[64 * j + dx + wl_idx, blk, :, p, :, wl_idx, :] = (
                        src.transpose(2, 0, 1, 3)
                    )
        f_hosts.append(F_host)
        b_hosts.append(B_host.reshape(128, NBLK, HC, NP3, NCOL))
    return f_hosts, b_hosts


# ---------------- v5: 2D-banded tiles, all 25 taps in one K=128 matmul ----------------
# Output tiled into (DT=5 low-res rows) x (LW=10 low-res cols) tiles; per tile the
# contraction dim packs the full 2D source window: K = (DT+4 rows) x (LW+4 cols)
# = 9*14 = 126 (pad 128). One matmul per (ct, t, u): N = DT*2*LW*2 = 200 output
# pixels, no PSUM accumulation chains. 100 matmuls/core instead of 500, input
# traffic 5.8 MB (vs 8.3), output fp16 (5.1 MB vs 10.2 f32; host casts back).
DT5 = 5            # low-res rows per tile
LW5 = 10           # low-res cols per tile
NT5 = HC // DT5    # 5 h-tiles per core
NU5 = W // LW5     # 10 w-tiles
KR5 = DT5 + KS - 1   # 9 source rows per tile
KW5 = LW5 + KS - 1   # 14 source cols per tile (K = 9*14 = 126)
NC5 = DT5 * S * LW5 * S  # 200 output pixels per tile
NTU5 = NT5 * NU5   # 50 tiles


def build_program_v5(iters: int = 1, dt=F16, odt=F16, chunks: int = 5,
                     obufs: int = 3, psbufs: int = 8, parts: str = "full",
                     in_eng: str = "sync", in_eng2: str | None = None,
                     out_eng: str = "sync", copy_pat: str = "vs",
                     upair: int = 2, tbatch: int = 1, stagger: bool = False):
    """Knobs: in_eng/in_eng2 (F/B) and out_eng pick the DMA-issuing engine
    (sync and scalar have independent HWDGE rings; others go via SWDGE);
    copy_pat cycles PSUM->SBUF copy engines ('v'=vector, 's'=scalar); upair
    packs that many u-tiles into one PSUM bank (fewer, larger copies).

    Device output layout is the raw PSUM order [C, t, u0, (k,h',a,w,b)];
    the host permutes back (see post_v5). All engine copies are contiguous.
    """
    nc = bacc.Bacc(None, target_bir_lowering=False, debug=False)
    f_in = nc.dram_tensor("f", [128, NTU5, C], dt, kind="ExternalInput")
    b_in = nc.dram_tensor("b", [128, NTU5, NC5], dt, kind="ExternalInput")
    npair = NU5 // upair
    out = nc.dram_tensor("out", [C, NT5, npair * upair * NC5], odt,
                         kind="ExternalOutput")

    with tile.TileContext(nc) as tc:
        with (
            tc.tile_pool(name="fsb", bufs=1) as fpool,
            tc.tile_pool(name="bsb", bufs=1) as bpool,
            tc.tile_pool(name="osb", bufs=obufs) as opool,
            tc.tile_pool(name="ps", bufs=psbufs, space="PSUM") as pspool,
        ):
            ieng = getattr(nc, in_eng)
            ieng2 = getattr(nc, in_eng2) if in_eng2 else ieng
            oengs = (
                [nc.sync, nc.scalar]
                if out_eng == "alt"
                else [getattr(nc, out_eng)]
            )

            def body(_=None):
                F_sb = fpool.tile([128, NTU5, C], dt)
                B_sb = bpool.tile([128, NTU5, NC5], dt)
                if parts != "nodmain":
                    if isinstance(chunks, int):
                        step = (NTU5 + chunks - 1) // chunks
                        plan = [step] * chunks
                    else:
                        plan = list(chunks)
                    c0 = 0
                    for sz in plan:
                        c1 = min(c0 + sz, NTU5)
                        if c1 > c0:
                            ieng.dma_start(F_sb[:, c0:c1], f_in[:, c0:c1])
                            ieng2.dma_start(B_sb[:, c0:c1], b_in[:, c0:c1])
                        c0 = c1
                if parts == "dmain":
                    return
                ei = 0
                for ct in range(2):
                    for t0 in range(0, NT5, tbatch):
                        t1 = min(t0 + tbatch, NT5)
                        osb = opool.tile(
                            [128, (t1 - t0) * npair, upair * NC5], odt
                        )
                        for t in range(t0, t1):
                            for u0 in range(0, NU5, upair):
                                ps = pspool.tile([128, upair * NC5], F32,
                                                 name=f"ps_{ct}_{t}_{u0}",
                                                 tag="ps")
                                for k in range(upair):
                                    tu = t * NU5 + u0 + k
                                    nc.tensor.matmul(
                                        ps[:, k * NC5 : (k + 1) * NC5],
                                        F_sb[:, tu, ct * 128 : (ct + 1) * 128],
                                        B_sb[:, tu, :],
                                        start=(k == 0),
                                        stop=(k == upair - 1),
                                    )
                                if parts == "nocopy":
                                    continue
                                dst = osb[:, (t - t0) * npair + u0 // upair, :]
                                eng = copy_pat[ei % len(copy_pat)]
                                if eng == "v":
                                    nc.vector.tensor_copy(dst, ps[:])
                                else:
                                    nc.scalar.copy(dst, ps[:])
                                ei += 1
                        if parts == "nocopy":
                            continue
                        oengs[(ct * NT5 + t0) % len(oengs)].dma_start(
                            out[ct * 128 : (ct + 1) * 128, t0:t1, :],
                            osb[:].rearrange("p n c -> p (n c)"),
                        )

            if iters == 1:
                body()
            else:
                with tc.For_i(0, iters, 1, staggered_reset=stagger) as _i:
                    body(_i)
    nc.finalize()
    return nc


def post_v5(res_out: np.ndarray, upair: int = 2) -> np.ndarray:
    """Undo the device output permutation: [C, t, u0, k, h', a, w, b] ->
    [C, (t, h', a), (u0, k, w, b)] = [C, 50, 200] for one core's quarter."""
    npair = NU5 // upair
    arr = res_out.reshape(C, NT5, npair, upair, DT5, 2, LW5, 2)
    return (
        arr.transpose(0, 1, 4, 5, 2, 3, 6, 7)
        .reshape(C, S * HC, S * W)
        .astype(np.float32)
    )


def host_prep_v5(features: np.ndarray, masks: np.ndarray):
    """v5 layouts. Per core (n, q):
    f: [128, 50, 256]  f[p=(rho*14+wp), (t,u), c] = fpad[c, 25q+5t+rho, 10u+wp]
    b: [128, 50, 200]  b[p, (t,u), (h',a,w,b)] = m[dy=rho-h', dx=wp-w, ...] (2D band)
    """
    padded = np.pad(features, ((0, 0), (0, 0), (R, R), (R, R))).astype(np.float16)
    hh = np.repeat(np.arange(DT5), LW5)   # h' per (h', w) pair
    ww = np.tile(np.arange(LW5), DT5)     # w
    f_hosts, b_hosts = [], []
    for core in range(8):
        n, q = divmod(core, 4)
        h0 = HC * q
        pn = padded[n]  # [C, 104, 104]
        F = np.zeros((128, NT5, NU5, C), np.float16)
        for rho in range(KR5):
            for wp in range(KW5):
                sl = pn[:, h0 + rho : h0 + rho + HC : DT5, wp : wp + W : LW5]
                F[rho * KW5 + wp] = sl.transpose(1, 2, 0)
        # masks[n]: [25, 200, 200] -> [dy, dx, h, a, w, b]
        m6 = masks[n].reshape(KS, KS, H, S, W, S)[:, :, h0 : h0 + HC]
        m6 = m6.astype(np.float16)
        B = np.zeros((128, NT5, NU5, DT5, 2, LW5, 2), np.float16)
        for dy in range(KS):
            for dx in range(KS):
                pp = (hh + dy) * KW5 + (ww + dx)
                # src [t, h', a, u, w, b]
                src = m6[dy, dx].reshape(NT5, DT5, 2, NU5, LW5, 2)
                # fancy: result dims [pair, t, u, a, b]
                B[pp, :, :, hh, :, ww, :] = src[:, hh, :, :, ww, :].transpose(
                    0, 1, 3, 2, 4
                )
        f_hosts.append(F.reshape(128, NTU5, C))
        b_hosts.append(B.reshape(128, NTU5, NC5))
    return f_hosts, b_hosts


_NC_CACHE = {}

# Best measured config: two input DMA chunks (~1.6 MB each, ~375 GB/s),
# output DMAs on the scalar engine's HWDGE ring so they overlap the input
# stream on the sync ring, copies split 50/50 vector/scalar.
BEST_KW = dict(chunks=(10, 40), out_eng="scalar")
BUILD = build_program_v5
PREP = host_prep_v5


def _get_program(iters: int = 1):
    # v5: 2D-banded single-matmul-per-tile formulation (see build_program_v5).
    if iters not in _NC_CACHE:
        _NC_CACHE[iters] = BUILD(iters, **BEST_KW)
    return _NC_CACHE[iters]


def kernel(features: np.ndarray, masks: np.ndarray) -> np.ndarray:
    features = np.ascontiguousarray(features, dtype=np.float32)
    masks = np.ascontiguousarray(masks, dtype=np.float32)
    f_hosts, b_hosts = PREP(features, masks)
    in_maps = [{"f": f_hosts[c], "b": b_hosts[c]} for c in range(8)]
    nc = _get_program(1)
    res = run_bass_kernel_spmd(nc, in_maps, list(range(8)))
    out = np.empty((N, C, S * H, S * W), np.float32)
    for core in range(8):
        n, q = divmod(core, 4)
        out[n, :, S * HC * q : S * HC * (q + 1), :] = post_v5(
            res.results[core]["out"]
        )
    return out

